# revision 20
# baseline (speedup 1.0000x reference)
"""Causal scaled-dot attention (with QKV projections) for TRN2, 8 NeuronCores.

Data-parallel across the batch dim: B=16 batches -> 2 per core. Each core:
  q = X_q @ Wq.T + bq ; k = X_k @ Wk.T + bk ; v = X_v @ Wv.T + bv
  scores = q @ k.T / sqrt(H), causal-masked softmax, ctx = attn @ v
Outputs: ctx [B,S,H] f32 and attn.T [B,S,Q] f32.

All matmuls run in bf16 (fp32 PSUM accumulation). Causal structure is
exploited: only lower-triangular score blocks are computed (exp of the
reference's -1e7 mask underflows to exactly 0 in fp32, so skipping masked
blocks and writing zeros is exact).

Pipeline structure (per core):
  - W: load f32, cast bf16, transpose on PE via identity matmuls -> wT
  - per batch, per 512-row chunk: X load/cast/store-bf16-scratch,
    xbar-transposed reload (hin on partitions), then QKV projection matmuls
    (q/k biases folded into the PSUM->SBUF eviction on ScalarE, v bias
    folded into a DVE eviction-add of a replicated bias tile)
  - attention, software-pipelined across 128-row q-blocks: scores(qb+1) is
    emitted before transpose/AV(qb) so PE never waits on the softmax chain
"""

import math
import os
from contextlib import ExitStack

import numpy as np

import concourse.bacc as bacc
import concourse.bass as bass
import concourse.mybir as mybir
import concourse.tile as tile
from concourse.bass_utils import run_bass_kernel_spmd
from concourse.masks import make_causal_mask, make_identity

F32 = mybir.dt.float32
BF16 = mybir.dt.bfloat16
P = 128
H = 1024
KB = H // P  # h blocks (contraction)
MASK_VAL = -1.0e9


def build_nc(n_batch=2, s_len=2048):
    TB = s_len // P       # 128-row t/s blocks per batch
    TC = s_len // 512     # 512-row t chunks per batch
    QB = TB               # q blocks per batch
    HC = H // 512         # 512-col h chunks
    scale = 1.0 / math.sqrt(H)

    nc = bacc.Bacc("TRN2", target_bir_lowering=False)

    xq = nc.dram_tensor("queries", [n_batch, s_len, H], F32, kind="ExternalInput")
    xk = nc.dram_tensor("keys", [n_batch, s_len, H], F32, kind="ExternalInput")
    xv = nc.dram_tensor("values", [n_batch, s_len, H], F32, kind="ExternalInput")
    wq = nc.dram_tensor("Wq", [H, H], F32, kind="ExternalInput")
    wk = nc.dram_tensor("Wk", [H, H], F32, kind="ExternalInput")
    wv = nc.dram_tensor("Wv", [H, H], F32, kind="ExternalInput")
    bq = nc.dram_tensor("bq", [H], F32, kind="ExternalInput")
    bk = nc.dram_tensor("bk", [H], F32, kind="ExternalInput")
    bv = nc.dram_tensor("bv", [H], F32, kind="ExternalInput")
    ctx_out = nc.dram_tensor("ctx", [n_batch, s_len, H], F32, kind="ExternalOutput")
    att_out = nc.dram_tensor("att", [n_batch, s_len, s_len], F32, kind="ExternalOutput")

    x_in = {"q": xq, "k": xk, "v": xv}
    w_in = {"q": wq, "k": wk, "v": wv}
    b_in = {"q": bq, "k": bk, "v": bv}

    with tile.TileContext(nc) as tc, ExitStack() as ctx:
        const = ctx.enter_context(tc.tile_pool(name="const", bufs=1))
        big = ctx.enter_context(tc.tile_pool(name="big", bufs=1))
        work = ctx.enter_context(tc.tile_pool(name="work", bufs=2))
        dram = ctx.enter_context(tc.tile_pool(name="dram", bufs=1, space="DRAM"))
        pp_proj = ctx.enter_context(tc.tile_pool(name="pp_proj", bufs=2, space="PSUM"))
        pp_sc = ctx.enter_context(tc.tile_pool(name="pp_sc", bufs=2, space="PSUM"))
        pp_att = ctx.enter_context(tc.tile_pool(name="pp_att", bufs=2, space="PSUM"))
        pp_av = ctx.enter_context(tc.tile_pool(name="pp_av", bufs=2, space="PSUM"))

        # ---- constants -----------------------------------------------------
        ident = const.tile([P, P], BF16)
        make_identity(nc, ident)
        cmask = const.tile([P, P], F32)
        make_causal_mask(nc, cmask, mask_val=MASK_VAL)
        ones = const.tile([1, P], BF16)
        nc.vector.memset(ones, 1.0)
        zeros = const.tile([P, 4 * P], F32)
        nc.vector.memset(zeros, 0.0)

        # per-partition bias columns for q/k evictions: [128, KB] f32
        bias_sb = {}
        for name in ("q", "k"):
            bcol = const.tile([P, KB], F32, name=f"bias_{name}")
            nc.scalar.dma_start(bcol, b_in[name].rearrange("(o p) -> p o", p=P))
            bias_sb[name] = bcol

        # bv replicated across partitions: [128, H] f32 (via K=1 matmuls)
        bv_f32 = work.tile([1, H], F32, tag="bv_f32", bufs=1)
        nc.scalar.dma_start(bv_f32, bv.rearrange("(a b) -> a b", a=1))
        bv_bf = const.tile([1, H], BF16)
        nc.vector.tensor_copy(out=bv_bf, in_=bv_f32)
        bv_rep = const.tile([P, H], F32)
        for hc in range(HC):
            psb = pp_att.tile([P, 512], F32, tag="ps_att", name=f"psbv_{hc}")
            nc.tensor.matmul(psb, ones[0:1, :P], bv_bf[0:1, hc * 512:(hc + 1) * 512],
                             start=True, stop=True)
            nc.vector.tensor_copy(out=bv_rep[:, hc * 512:(hc + 1) * 512], in_=psb)

        # ---- weights: load f32, cast bf16, PE-transpose into wT ------------
        # wT[name]: [128 hin_p, KB hin_o, H hout] bf16
        wT = {name: const.tile([P, KB, H], BF16, name=f"wT_{name}")
              for name in ("q", "k", "v")}

        def w_prep(name):
            for r in range(KB):  # hout blocks
                win = work.tile([P, H], F32, tag="xin", bufs=4,
                                name=f"win_{name}_{r}")
                nc.scalar.dma_start(win, w_in[name][r * P:(r + 1) * P, :])
                wbf = work.tile([P, H], BF16, tag="xbf", name=f"wbf_{name}_{r}")
                nc.vector.tensor_copy(out=wbf, in_=win)
                for g in range(KB // 4):  # transpose 4 hin-blocks per psum
                    pst = pp_att.tile([P, 512], F32, tag="ps_att",
                                      name=f"pswt_{name}_{r}_{g}")
                    for j in range(4):
                        hi = g * 4 + j
                        nc.tensor.matmul(
                            pst[:, j * P:(j + 1) * P],
                            wbf[:, hi * P:(hi + 1) * P], ident,
                            start=True, stop=True, skip_group_check=True)
                    nc.scalar.copy(
                        out=wT[name][:, g * 4:g * 4 + 4, r * P:(r + 1) * P],
                        in_=pst.rearrange("p (j q) -> p j q", q=P))

        def make_xt(b, tc_i, name):
            # X chunk -> bf16 -> PE-transpose -> [128 hin_p, KB hin_o, 512 t]
            t0 = tc_i * 512
            x_t = work.tile([P, KB, 512], BF16, tag="xt", bufs=3,
                            name=f"xt_{name}_{b}_{tc_i}")
            for tb in range(4):
                xin = work.tile([P, H], F32, tag="xin", bufs=4,
                                name=f"xin_{name}_{b}_{tc_i}_{tb}")
                eng = nc.sync if tb % 2 == 0 else nc.scalar
                eng.dma_start(
                    xin, x_in[name][b, t0 + tb * P:t0 + (tb + 1) * P, :])
                xbf = work.tile([P, H], BF16, tag="xbf",
                                name=f"xbf_{name}_{b}_{tc_i}_{tb}")
                nc.vector.tensor_copy(out=xbf, in_=xin)
                for g in range(KB // 4):
                    pst = pp_proj.tile([P, 512], F32, tag="ps_proj",
                                       name=f"psxt_{name}_{b}_{tc_i}_{tb}_{g}")
                    for j in range(4):
                        hi = g * 4 + j
                        nc.tensor.matmul(
                            pst[:, j * P:(j + 1) * P],
                            xbf[:, hi * P:(hi + 1) * P], ident,
                            start=True, stop=True, skip_group_check=True)
                    dst = x_t[:, g * 4:g * 4 + 4, tb * P:(tb + 1) * P]
                    src = pst.rearrange("p (j q) -> p j q", q=P)
                    if (tb * (KB // 4) + g) % 2 == 0:
                        nc.vector.tensor_copy(out=dst, in_=src)
                    else:
                        nc.scalar.copy(out=dst, in_=src)
            return x_t

        def proj_qk(name, b, tc_i, x_t, kt_sb, qt_scr):
            t0 = tc_i * 512
            for ho in range(KB):
                ps = pp_proj.tile([P, 512], F32, tag="ps_proj",
                                  name=f"psp_{name}_{b}_{tc_i}_{ho}")
                for kb in range(KB):
                    nc.tensor.matmul(
                        ps, wT[name][:, kb, ho * P:(ho + 1) * P],
                        x_t[:, kb, :],
                        start=(kb == 0), stop=(kb == KB - 1))
                if name == "k":
                    nc.scalar.add(out=kt_sb[:, ho, t0:t0 + 512], in_=ps,
                                  add=bias_sb["k"][:, ho:ho + 1])
                else:
                    qst = work.tile([P, 512], BF16, tag="qst",
                                    name=f"qst_{b}_{tc_i}_{ho}")
                    nc.scalar.add(out=qst, in_=ps,
                                  add=bias_sb["q"][:, ho:ho + 1])
                    nc.scalar.dma_start(qt_scr[ho, :, t0:t0 + 512], qst)

        def proj_v(b, tc_i, x_t, v_sb):
            for tb in range(4):
                for hc in range(HC):
                    ps = pp_proj.tile([P, 512], F32, tag="ps_proj",
                                      name=f"psv_{b}_{tc_i}_{tb}_{hc}")
                    for kb in range(KB):
                        nc.tensor.matmul(
                            ps, x_t[:, kb, tb * P:(tb + 1) * P],
                            wT["v"][:, kb, hc * 512:(hc + 1) * 512],
                            start=(kb == 0), stop=(kb == KB - 1))
                    nc.vector.tensor_tensor(
                        v_sb[:, tc_i * 4 + tb, hc * 512:(hc + 1) * 512],
                        ps, bv_rep[:, hc * 512:(hc + 1) * 512],
                        mybir.AluOpType.add)

        w_prep("q")  # wk/wv prep is interleaved into batch-0 chunk-0 below

        for b in range(n_batch):
            kt_sb = big.tile([P, KB, s_len], BF16, tag="kt", name=f"kt_{b}")
            v_sb = big.tile([P, TB, H], BF16, tag="v", name=f"v_{b}")
            qt_scr = dram.tile([KB, P, s_len], BF16, tag="qt_scr", bufs=2,
                               name=f"qt_scr_{b}")
            for tc_i in range(TC):
                if b == 0 and tc_i == 0:
                    # startup: emit in data-arrival order so PE starts ASAP
                    x_t = make_xt(0, 0, "q")
                    proj_qk("q", 0, 0, x_t, kt_sb, qt_scr)
                    w_prep("k")
                    x_t = make_xt(0, 0, "k")
                    proj_qk("k", 0, 0, x_t, kt_sb, qt_scr)
                    w_prep("v")
                    x_t = make_xt(0, 0, "v")
                    proj_v(0, 0, x_t, v_sb)
                else:
                    for name in ("q", "k"):
                        x_t = make_xt(b, tc_i, name)
                        proj_qk(name, b, tc_i, x_t, kt_sb, qt_scr)
                    x_t = make_xt(b, tc_i, "v")
                    proj_v(b, tc_i, x_t, v_sb)

            # ---- phase C: attention, software-pipelined over q blocks ------
            stage1_out = {}

            def stage1(qb):
                L = (qb + 1) * P
                nch = (L + 511) // 512
                qt_blk = work.tile([P, KB, P], BF16, tag="qt_blk",
                                   name=f"qtb_{b}_{qb}")
                nc.scalar.dma_start(
                    qt_blk,
                    qt_scr[:, :, qb * P:(qb + 1) * P].rearrange("o p t -> p o t"))

                exp_bf = work.tile([P, s_len], BF16, tag="exp", name=f"exp_{b}_{qb}")
                acc = work.tile([P, 4], F32, tag="acc", name=f"acc_{b}_{qb}")
                for sc in range(nch):
                    s0 = sc * 512
                    w = min(512, L - s0)
                    ps = pp_sc.tile([P, 512], F32, tag="ps_sc",
                                    name=f"pssc_{b}_{qb}_{sc}")
                    for kb in range(KB):
                        nc.tensor.matmul(
                            ps[:, :w], qt_blk[:, kb, :], kt_sb[:, kb, s0:s0 + w],
                            start=(kb == 0), stop=(kb == KB - 1))
                    if s0 + w == L:  # chunk contains the diagonal block
                        d0 = w - P
                        nc.vector.tensor_tensor(
                            ps[:, d0:d0 + P], ps[:, d0:d0 + P], cmask,
                            mybir.AluOpType.add)
                    nc.scalar.activation(
                        out=exp_bf[:, s0:s0 + w], in_=ps[:, :w],
                        func=mybir.ActivationFunctionType.Exp,
                        scale=scale, accum_out=acc[:, sc:sc + 1])

                recip = work.tile([P, 1], F32, tag="recip", name=f"rcp_{b}_{qb}")
                if nch > 1:
                    rsum = work.tile([P, 1], F32, tag="rsum", name=f"rs_{b}_{qb}")
                    nc.vector.tensor_reduce(
                        rsum, acc[:, :nch], mybir.AxisListType.X,
                        mybir.AluOpType.add)
                    nc.vector.reciprocal(recip, rsum)
                else:
                    nc.vector.reciprocal(recip, acc[:, 0:1])

                attn_bf = work.tile([P, s_len], BF16, tag="attn",
                                    name=f"attn_{b}_{qb}")
                nc.vector.tensor_scalar_mul(attn_bf[:, :L], exp_bf[:, :L], recip)
                stage1_out[qb] = attn_bf

            def stage2(qb):
                L = (qb + 1) * P
                attn_bf = stage1_out.pop(qb)
                attnT_bf = work.tile([P, TB, P], BF16, tag="attnT", bufs=1,
                                     name=f"attnT_{b}_{qb}")
                for g in range((qb + 1 + 3) // 4):
                    nj = min(4, qb + 1 - g * 4)
                    pst = pp_att.tile([P, 512], F32, tag="ps_att",
                                      name=f"psat_{b}_{qb}_{g}")
                    for j in range(nj):
                        sb = g * 4 + j
                        nc.tensor.matmul(
                            pst[:, j * P:(j + 1) * P],
                            attn_bf[:, sb * P:(sb + 1) * P], ident,
                            start=True, stop=True, skip_group_check=True)
                    att_f32 = work.tile([P, 512], F32, tag="att_f32", bufs=3,
                                        name=f"atf_{b}_{qb}_{g}")
                    nc.scalar.copy(out=att_f32[:, :nj * P], in_=pst[:, :nj * P])
                    nc.scalar.dma_start(
                        att_out[b][g * 512:g * 512 + nj * P, qb * P:(qb + 1) * P]
                        .rearrange("(j p) q -> p j q", p=P),
                        att_f32[:, :nj * P].rearrange("p (j q) -> p j q", q=P))
                    nc.vector.tensor_copy(
                        out=attnT_bf[:, g * 4:g * 4 + nj, :],
                        in_=att_f32[:, :nj * P].rearrange("p (j q) -> p j q", q=P))

                # masked-region zeros for ALL batches, deferred to the last
                # batch's attention window (bulk DMA queue is idle there)
                if b == n_batch - 1:
                    nblk = TB - 1 - qb
                    for bz in range(n_batch):
                        for z0 in range(0, nblk, 4):
                            zn = min(4, nblk - z0)
                            nc.sync.dma_start(
                                att_out[bz][L + z0 * P:L + (z0 + zn) * P,
                                            qb * P:(qb + 1) * P]
                                .rearrange("(j p) q -> p j q", p=P),
                                zeros[:, :zn * P].rearrange("p (j q) -> p j q", q=P))

                # AV: ctx[q 128, h 512] = sum_sb attnT[:,sb,:]^T @ v[:,sb,hc]
                for hc in range(HC):
                    psa = pp_av.tile([P, 512], F32, tag="ps_av",
                                     name=f"psav_{b}_{qb}_{hc}")
                    for sb in range(qb + 1):
                        nc.tensor.matmul(
                            psa, attnT_bf[:, sb, :],
                            v_sb[:, sb, hc * 512:(hc + 1) * 512],
                            start=(sb == 0), stop=(sb == qb))
                    ctx_sb = work.tile([P, 512], F32, tag="ctx_sb", bufs=3,
                                       name=f"ctxs_{b}_{qb}_{hc}")
                    nc.vector.tensor_copy(out=ctx_sb, in_=psa)
                    nc.scalar.dma_start(
                        ctx_out[b][qb * P:(qb + 1) * P, hc * 512:(hc + 1) * 512],
                        ctx_sb)

            stage1(0)
            for qb in range(1, QB):
                stage1(qb)
                stage2(qb - 1)
            stage2(QB - 1)

    nc.compile()
    return nc


_NC_CACHE = {}


def _get_nc(n_batch, s_len):
    key = (n_batch, s_len)
    if key not in _NC_CACHE:
        _NC_CACHE[key] = build_nc(n_batch, s_len)
    return _NC_CACHE[key]


def kernel(queries, keys, values, Wq, bq, Wk, bk, Wv, bv):
    queries = np.asarray(queries, dtype=np.float32)
    keys = np.asarray(keys, dtype=np.float32)
    values = np.asarray(values, dtype=np.float32)
    Wq = np.asarray(Wq, dtype=np.float32)
    Wk = np.asarray(Wk, dtype=np.float32)
    Wv = np.asarray(Wv, dtype=np.float32)
    bq = np.asarray(bq, dtype=np.float32)
    bk = np.asarray(bk, dtype=np.float32)
    bv = np.asarray(bv, dtype=np.float32)

    B, S, Hh = queries.shape
    n_cores = 8
    assert B % n_cores == 0
    nb = B // n_cores

    nc = _get_nc(nb, S)

    in_maps = []
    for c in range(n_cores):
        sl = slice(c * nb, (c + 1) * nb)
        in_maps.append({
            "queries": queries[sl], "keys": keys[sl], "values": values[sl],
            "Wq": Wq, "Wk": Wk, "Wv": Wv, "bq": bq, "bk": bk, "bv": bv,
        })

    trace = bool(int(os.environ.get("KERNEL_TRACE", "0")))
    res = None
    for attempt in range(3):
        try:
            res = run_bass_kernel_spmd(
                nc, in_maps, core_ids=list(range(n_cores)), trace=trace)
            break
        except Exception:
            # transient NRT_EXEC_UNIT_UNRECOVERABLE has been observed on a
            # freshly-acquired device; retry once or twice before giving up
            if attempt == 2:
                raise
    kernel.last_results = res
    kernel.last_exec_time_ns = res.exec_time_ns

    ctx = np.concatenate([r["ctx"] for r in res.results], axis=0)
    att = np.concatenate([r["att"] for r in res.results], axis=0)
    return ctx, att


if __name__ == "__main__":
    np.random.seed(0)
    nb, S = 1, 512
    ins = {
        "queries": np.random.randn(nb, S, H).astype(np.float32),
        "keys": np.random.randn(nb, S, H).astype(np.float32),
        "values": np.random.randn(nb, S, H).astype(np.float32),
    }
    for n in ("q", "k", "v"):
        ins[f"W{n}"] = (np.random.randn(H, H) / 32).astype(np.float32)
        ins[f"b{n}"] = (np.random.randn(H) / 32).astype(np.float32)

    nc = build_nc(nb, S)
    in_map = dict(ins)
    res = run_bass_kernel_spmd(nc, [in_map], core_ids=[0])
    ctx_a, att_a = res.results[0]["ctx"], res.results[0]["att"]

    # numpy reference
    q = ins["queries"] @ ins["Wq"].T + ins["bq"]
    k = ins["keys"] @ ins["Wk"].T + ins["bk"]
    v = ins["values"] @ ins["Wv"].T + ins["bv"]
    sc = np.einsum("bqh,bsh->bqs", q, k) / np.sqrt(H)
    mask = np.where(np.arange(S)[None, :] > np.arange(S)[:, None], -1e7, 0.0)
    sc = sc + mask[None]
    sc = sc - sc.max(-1, keepdims=True)
    e = np.exp(sc)
    attn = e / e.sum(-1, keepdims=True)
    ctx_e = np.einsum("bqs,bsh->bqh", attn, v)
    att_e = attn.transpose(0, 2, 1)

    for nme, a, e_ in (("ctx", ctx_a, ctx_e), ("att", att_a, att_e)):
        rel = np.linalg.norm(a - e_) / np.linalg.norm(e_)
        print(f"{nme}: rel={rel:.3e} max_abs={np.abs(a - e_).max():.3e}")
